# revision 9
# baseline (speedup 1.0000x reference)
"""ECE loss kernel v2 for Trainium2, data-parallel over 8 NeuronCores.

Host: shards 2M samples contiguously across 8 cores, rolls each row so the
label sits at column 0 (softmax/binning are permutation invariant), casts to
bf16 (halves HBM traffic; rel-err budget 2e-2 >> bf16 noise).

Device per pair-tile (8192 samples, [128, 6400] bf16):
  ScalarE: one exp over the whole tile.
  DVE:     pairwise bf16 collapse trees (2x perf mode) for per-sample
           sum/max, short 1x reduces to finish, reciprocal, conf/acc pack,
           cumulative threshold masks.
  PE:      8 block-diagonal matmuls accumulate per-bin (sum_conf, sum_acc)
           into PSUM across all tiles.
  Final:   diag extract, cum-diff, 15-bin AllReduce, ECE = sum|dS-dA|/N.
"""

import dataclasses
import sys

import numpy as np

sys.path.insert(0, "/opt/trn_rl_repo")

from concourse import bacc, bass, mybir, tile  # noqa: E402
from concourse import bass_utils  # noqa: E402

P = 128
C = 100
SPP = 64          # sample-slots per tile (groups per partition)
TILE = P * SPP    # samples per tile = 8192
NBINS = 15
N_CORES = 8
N_TOTAL = 2_000_000
BIG = 80.0
G_MM = 8          # groups per hist matmul block (diag blocks)

F32 = mybir.dt.float32
BF16 = mybir.dt.bfloat16
AX = mybir.AxisListType
ALU = mybir.AluOpType
ACTF = mybir.ActivationFunctionType

# knobs
GPSIMD_MASK = False   # generic TENSOR_TENSOR is not in the Pool ISA
GPSIMD_ACC = False
DEEP_L2 = True
RECIP_FAST = True     # reciprocal_approx_fast instead of exact reciprocal
MIXED_CONF = True     # conf = mx(bf16) * rd(f32) directly, no bf16 cast
SE_CONF_REP = True    # replicate conf x15 on ScalarE so mask compare hits 2x


def _bcast(ap, extra):
    return dataclasses.replace(ap, ap=ap.ap + [[0, extra]])


def build_program(T: int, n_total: int, n_cores: int = N_CORES):
    nc = bacc.Bacc("TRN2", target_bir_lowering=False, debug=False,
                   num_devices=n_cores)

    logits_d = nc.dram_tensor("logits", [T * P, SPP * C], BF16,
                              kind="ExternalInput")
    tempr_d = nc.dram_tensor("tempr", [P, 1], F32, kind="ExternalInput")
    thr_d = nc.dram_tensor("thr", [P, SPP * NBINS], BF16, kind="ExternalInput")
    wvec_d = nc.dram_tensor("wvec", [2, 1], F32, kind="ExternalInput")
    out_d = nc.dram_tensor("out", [2, NBINS], F32, kind="ExternalOutput")

    NMM = SPP // G_MM  # matmuls per pair-tile

    with tile.TileContext(nc) as tc:
        with (
            tc.tile_pool(name="const", bufs=1) as const,
            tc.tile_pool(name="rawp", bufs=3) as rawp,
            tc.tile_pool(name="ep", bufs=2) as ep,
            tc.tile_pool(name="sb", bufs=2) as sbp,
            tc.tile_pool(name="fin", bufs=1) as fin,
            tc.tile_pool(name="psH", bufs=1, space="PSUM") as psH,
            tc.tile_pool(name="psF", bufs=1, space="PSUM") as psF,
            tc.tile_pool(name="dram", bufs=1, space="DRAM") as dram,
        ):
            tempr_t = const.tile([P, 1], F32)
            nc.sync.dma_start(tempr_t, tempr_d.ap())
            thr_t = const.tile([P, SPP * NBINS], BF16)
            nc.sync.dma_start(thr_t, thr_d.ap())
            wvec_t = const.tile([2, 1], F32)
            nc.sync.dma_start(wvec_t, wvec_d.ap())
            invT = const.tile([P, 1], F32)
            nc.vector.reciprocal(invT, tempr_t)
            thr3 = thr_t.rearrange("p (g b) -> p g b", g=SPP)

            hist = psH.tile([2 * G_MM, G_MM * NBINS], F32)

            logits_ap = logits_d.ap()
            for t in range(T):
                raw = rawp.tile([P, SPP * C], BF16, tag="raw", name="raw")
                nc.sync.dma_start(raw, logits_ap[t * P:(t + 1) * P, :])
                raw3 = raw.rearrange("p (g c) -> p g c", g=SPP)

                E = ep.tile([P, SPP * C], BF16, tag="E", name="E")
                nc.scalar.activation(E, raw, ACTF.Exp, scale=invT)
                E3 = E.rearrange("p (g c) -> p g c", g=SPP)

                # max path first: depends only on the raw DMA, so the DVE
                # can start it while ScalarE is still computing exp.
                m1 = sbp.tile([P, SPP * 50], BF16, tag="m1", name="m1")
                m13 = m1.rearrange("p (g c) -> p g c", g=SPP)
                nc.vector.tensor_tensor(m13, raw3[:, :, 0:50],
                                        raw3[:, :, 50:100], op=ALU.max)
                m2 = sbp.tile([P, SPP * 25], BF16, tag="m2", name="m2")
                m23 = m2.rearrange("p (g c) -> p g c", g=SPP)
                nc.vector.tensor_tensor(m23, m13[:, :, 0:25], m13[:, :, 25:50],
                                        op=ALU.max)
                mx = sbp.tile([P, SPP], BF16, tag="mx", name="mx")
                # overlapping halves (idempotent, col 12 read twice)
                m3 = sbp.tile([P, SPP * 13], BF16, tag="m3", name="m3")
                m33 = m3.rearrange("p (g c) -> p g c", g=SPP)
                nc.vector.tensor_tensor(m33, m23[:, :, 0:13],
                                        m23[:, :, 12:25], op=ALU.max)
                nc.vector.reduce_max(mx, m33, axis=AX.X)

                s1 = sbp.tile([P, SPP * 50], BF16, tag="s1", name="s1")
                s13 = s1.rearrange("p (g c) -> p g c", g=SPP)
                nc.vector.tensor_tensor(s13, E3[:, :, 0:50], E3[:, :, 50:100],
                                        op=ALU.add)
                s2 = sbp.tile([P, SPP * 25], BF16, tag="s2", name="s2")
                s23 = s2.rearrange("p (g c) -> p g c", g=SPP)
                nc.vector.tensor_tensor(s23, s13[:, :, 0:25], s13[:, :, 25:50],
                                        op=ALU.add)
                D = sbp.tile([P, SPP], F32, tag="D", name="D")
                # sum: fold the low 12 cols onto the high 12 in place,
                # then reduce the contiguous 13-col tail [12:25].
                nc.vector.tensor_tensor(s23[:, :, 13:25], s23[:, :, 13:25],
                                        s23[:, :, 0:12], op=ALU.add)
                nc.vector.reduce_sum(D, s23[:, :, 12:25], axis=AX.X)

                emx = sbp.tile([P, SPP], BF16, tag="emx", name="emx")
                nc.scalar.activation(emx, mx, ACTF.Exp, scale=invT)
                rd = sbp.tile([P, SPP], F32, tag="rd", name="rd")
                if RECIP_FAST:
                    nc.vector.reciprocal_approx_fast(out=rd, in_=D)
                else:
                    nc.vector.reciprocal(rd, D)
                if MIXED_CONF:
                    rd_in = rd
                else:
                    rdb = sbp.tile([P, SPP], BF16, tag="rdb", name="rdb")
                    nc.vector.tensor_copy(rdb, rd)
                    rd_in = rdb

                pack = sbp.tile([P, 2 * SPP], BF16, tag="pack", name="pack")
                nc.vector.tensor_tensor(pack[:, 0:2 * SPP:2], emx, rd_in,
                                        op=ALU.mult)
                acc_eng = nc.gpsimd if GPSIMD_ACC else nc.vector
                acc_eng.tensor_tensor(pack[:, 1:2 * SPP:2], raw3[:, :, 0], mx,
                                      op=ALU.is_ge)

                mask = sbp.tile([P, SPP * NBINS], BF16, tag="mask", name="mask")
                mask3 = mask.rearrange("p (g b) -> p g b", g=SPP)
                conf_b = _bcast(pack[:, 0:2 * SPP:2], NBINS)
                if SE_CONF_REP:
                    # replicate conf x15 on the idle ScalarE so the DVE
                    # compare sees packed operands (2x perf mode).
                    conf_rep = sbp.tile([P, SPP * NBINS], BF16,
                                        tag="crep", name="crep")
                    nc.scalar.copy(conf_rep, conf_b)
                    crep3 = conf_rep.rearrange("p (g b) -> p g b", g=SPP)
                    nc.vector.tensor_tensor(mask3, crep3, thr3, op=ALU.is_gt)
                else:
                    mask_eng = nc.gpsimd if GPSIMD_MASK else nc.vector
                    mask_eng.tensor_tensor(mask3, conf_b, thr3, op=ALU.is_gt)

                for m in range(NMM):
                    nc.tensor.matmul(
                        hist,
                        lhsT=pack[:, 2 * G_MM * m:2 * G_MM * (m + 1)],
                        rhs=mask[:, NBINS * G_MM * m:NBINS * G_MM * (m + 1)],
                        start=(t == 0 and m == 0),
                        stop=(t == T - 1 and m == NMM - 1))

            # ---- finalize ----
            hist_sb = fin.tile([2 * G_MM, G_MM * NBINS], F32)
            nc.vector.tensor_copy(hist_sb, hist)
            stats = fin.tile([2, G_MM * NBINS], F32)
            for j in range(G_MM):
                eng = nc.sync if j % 2 == 0 else nc.scalar
                eng.dma_start(
                    stats[:, j * NBINS:(j + 1) * NBINS],
                    hist_sb[2 * j:2 * j + 2, j * NBINS:(j + 1) * NBINS])
            cum = fin.tile([2, NBINS], F32)
            nc.vector.reduce_sum(
                cum, stats.rearrange("p (j b) -> p b j", j=G_MM), axis=AX.X)
            cum16 = fin.tile([2, NBINS + 1], F32)
            nc.vector.memset(cum16, 0.0)
            nc.vector.tensor_copy(cum16[:, 0:NBINS], cum)
            bstats = fin.tile([2, NBINS], F32)
            nc.vector.tensor_tensor(bstats, cum16[:, 0:NBINS],
                                    cum16[:, 1:NBINS + 1], op=ALU.subtract)

            # per-core [2, NBINS] (sum_conf, sum_acc) per bin; host gathers
            # across cores and finishes ECE = sum_b |dS_b - dA_b| / N.
            nc.sync.dma_start(out_d.ap(), bstats)

    nc.compile()
    return nc


# ------------------------------------------------------------------- host

def build_core_slab(logits_f32: np.ndarray, labels: np.ndarray,
                    core: int, n_per_core: int, T: int) -> np.ndarray:
    """[T*P, SPP*C] bf16 slab for one core: row-rolled (label -> col 0),
    sample s of pair-tile t at partition s%P, group (s%TILE)//P."""
    import ml_dtypes
    lo = core * n_per_core
    x = logits_f32[lo:lo + n_per_core]
    lab = labels[lo:lo + n_per_core].astype(np.int64)
    # roll: out[i, j] = x[i, (lab[i] + j) % C]
    cols = (lab[:, None] + np.arange(C)[None, :]) % C
    rolled = np.take_along_axis(x, cols, axis=1).astype(ml_dtypes.bfloat16)
    n_pad = T * TILE - n_per_core
    if n_pad:
        pad = np.full((n_pad, C), -BIG, dtype=ml_dtypes.bfloat16)
        pad[:, 0] = BIG
        rolled = np.concatenate([rolled, pad], axis=0)
    # [T, SPP, P, C] -> [T, P, SPP, C]
    arr = rolled.reshape(T, SPP, P, C).transpose(0, 2, 1, 3)
    return np.ascontiguousarray(arr).reshape(T * P, SPP * C)


def make_const_inputs():
    import ml_dtypes
    thr = np.tile((np.arange(NBINS, dtype=np.float32) / np.float32(NBINS)),
                  SPP).astype(ml_dtypes.bfloat16)
    return {
        "thr": np.broadcast_to(thr, (P, SPP * NBINS)).copy(),
        "wvec": np.array([[1.0], [-1.0]], np.float32),
    }


_CACHE = {}


def _prepare(logits, labels, temperature, n_total, n_cores=N_CORES):
    n_per_core = n_total // n_cores
    T = -(-n_per_core // TILE)
    if T not in _CACHE:
        _CACHE[T] = build_program(T, n_total, n_cores)
    nc = _CACHE[T]

    logits = np.asarray(logits, dtype=np.float32)
    labels = np.asarray(labels)
    consts = make_const_inputs()
    tempr = np.broadcast_to(
        np.asarray(temperature, np.float32).ravel()[0:1], (P, 1)).copy()
    in_maps = []
    for c in range(n_cores):
        m = dict(consts)
        m["tempr"] = tempr
        m["logits"] = build_core_slab(logits, labels, c, n_per_core, T)
        in_maps.append(m)
    return nc, in_maps


def _ensure_ntff_hook():
    try:
        import antenv.axon_hooks  # noqa: F401
        return
    except ImportError:
        pass
    import types

    import antenv

    mod = types.ModuleType("antenv.axon_hooks")
    _hook = [None]
    mod.set_axon_ntff_profile_hook = lambda h: _hook.__setitem__(0, h)
    mod.get_axon_ntff_profile_hook = lambda: _hook[0]
    sys.modules["antenv.axon_hooks"] = mod
    antenv.axon_hooks = mod
    try:
        from trn_agent_boot.trn_boot import _ntff_profile_via_ctypes
        mod.set_axon_ntff_profile_hook(
            _ntff_profile_via_ctypes("/opt/axon/libaxon_pjrt.so"))
    except Exception:
        pass


def run(logits, labels, temperature, n_total=None, trace=False,
        n_cores=N_CORES):
    if trace:
        _ensure_ntff_hook()
    if n_total is None:
        n_total = int(np.asarray(labels).shape[0])
    nc, in_maps = _prepare(logits, labels, temperature, n_total, n_cores)
    res = bass_utils.run_bass_kernel_spmd(
        nc, in_maps, core_ids=list(range(n_cores)), trace=trace)
    bstats = np.zeros((2, NBINS), np.float64)
    for r in res.results:
        bstats += np.asarray(r["out"], dtype=np.float64)
    ece = np.abs(bstats[0] - bstats[1]).sum() / n_total
    out = np.asarray([ece], dtype=np.float32)
    return out, res


def kernel(logits, labels, temperature):
    out, _ = run(logits, labels, temperature)
    return out
